# revision 44
# baseline (speedup 1.0000x reference)
"""ViTDet-style windowed attention w/ decomposed rel-pos, on 8 TRN2 NeuronCores.

Problem: x(8,32,32,768), 12 heads, hd=64, N=1024 tokens per image.
Sharding: pure data-parallel over B — core b handles image b; weights/tables
replicated; no collectives.

Per-core math (all matmuls bf16, fp32 PSUM accumulation):
  qkv^T[o, n]   = qkv_wT.T-chunks @ x^T          (o on partitions, n free)
  scores^T[j,i] = K'^T.T @ Q'^T   per head, where the 128-deep contraction is
                  [k(64) | onehot_jh(32) | onehot_jw(32)] x
                  [q_s(64) | rel_h^T(32) | rel_w^T(32)]
                  -> q.k + rel_h[i,jh] + rel_w[i,jw] in ONE matmul
  E = exp(scores^T)               (no max-subtraction: |scores| < 3)
  [out^T; rowsum] = [v | 1].T @ E (ones column gives softmax denominator free)
  attn_outT = out^T * (1/rowsum)  (broadcast via DRAM-bounce partition bcast)
  final^T = proj_wT.T @ attn_outT + b_eff,  b_eff = proj_w@b_v + proj_b (host)

Row-parity trick: qkv^T psum tiles hold head 2t on partitions 0:64 and head
2t+1 on 64:128; DVE cannot shift partitions, so even heads keep q/k on rows
0:64 (rel parts 64:128) while odd heads keep q/k on rows 64:128 (rel parts
0:64). The augmented inner product is row-permutation invariant per head.
Scores psum tiles are 2-bank [128,1024] so one exp covers two matmuls.
"""

import numpy as np
import ml_dtypes

bf16 = ml_dtypes.bfloat16

B, H, W, C = 8, 32, 32, 768
NH, HD = 12, 64
N = H * W  # 1024
SCALE = HD ** -0.5

_NC = None


def _build(repeat=1):
    import concourse.mybir as mybir
    import concourse.tile as tile
    from concourse import bacc

    BF = mybir.dt.bfloat16
    F32 = mybir.dt.float32
    AF = mybir.ActivationFunctionType
    OP = mybir.AluOpType

    nc = bacc.Bacc(None, target_bir_lowering=False)

    xT_d = nc.declare_dram_parameter("xT", [128, 6, 1024], BF, isOutput=False)
    wqkv_d = nc.declare_dram_parameter("wqkv", [128, 6, 2304], BF, isOutput=False)
    wproj_d = nc.declare_dram_parameter("wproj", [64, 12, 768], BF, isOutput=False)
    bqk_d = nc.declare_dram_parameter("bqk", [128, 12], F32, isOutput=False)
    beff_d = nc.declare_dram_parameter("beff", [128, 6], F32, isOutput=False)
    rhT_d = nc.declare_dram_parameter("rhT", [64, 1024], BF, isOutput=False)
    rwT_d = nc.declare_dram_parameter("rwT", [64, 1024], BF, isOutput=False)
    eyeh_d = nc.declare_dram_parameter("eyeh", [32, 6, 1024], BF, isOutput=False)
    eyew_d = nc.declare_dram_parameter("eyew", [32, 6, 1024], BF, isOutput=False)
    out_d = nc.declare_dram_parameter("out", [768, 1024], F32, isOutput=True)

    with tile.TileContext(nc) as tc, \
            tc.tile_pool(name="consts", bufs=1) as consts, \
            tc.tile_pool(name="epool", bufs=3) as epool, \
            tc.tile_pool(name="ps", bufs=2, space="PSUM") as pspool, \
            tc.tile_pool(name="pa", bufs=4, space="PSUM") as papool, \
            tc.tile_pool(name="divp", bufs=3) as divp, \
            tc.tile_pool(name="drp", bufs=4, space="DRAM") as drp, \
            tc.tile_pool(name="outp", bufs=2) as outp:

        def emit():
            # xT shares its SBUF slot with attn_outT (tag "big"): xT dies
            # after phase 3, attn_outT is born in phase 4.
            xT = consts.tile([128, 6, 1024], BF, tag="big")
            wqkv = consts.tile([128, 6, 2304], BF)
            for ct in range(6):
                nc.sync.dma_start(out=xT[:, ct, :], in_=xT_d[:, ct, :])
                nc.sync.dma_start(out=wqkv[:, ct, :], in_=wqkv_d[:, ct, :])
            wproj = consts.tile([64, 12, 768], BF)
            bqk = consts.tile([128, 12], F32)
            nc.sync.dma_start(out=bqk, in_=bqk_d[:])
            beff = consts.tile([128, 6], F32)
            nc.sync.dma_start(out=beff, in_=beff_d[:])
            # DVE "touch" of DMA-loaded scalars: absorbs the DMA-lane
            # semaphore waits so downstream tensor_scalar ops need only the
            # PE wait (fewer event-semaphore splits).
            tch = consts.tile([1, 18], F32)
            nc.vector.tensor_copy(out=tch[:, 0:12], in_=bqk[0:1, :])
            nc.vector.tensor_copy(out=tch[:, 12:18], in_=beff[0:1, :])
            # rel-pos tables duplicated on both partition halves so odd-parity
            # matmuls (operands on partitions 64:128) have an aligned lhsT.
            rhT = consts.tile([128, 1024], BF)
            rwT = consts.tile([128, 1024], BF)

            KT = consts.tile([128, 12, 1024], BF)   # augmented K'^T per head
            QT = consts.tile([128, 12, 1024], BF)   # augmented Q'^T per head
            Vb = consts.tile([128, 8, 12, 65], BF)  # [n-part, jt, head, v|1]

            def even_heads(ap):  # [p, 12, i] -> [p, 6, i] (heads 0,2,..)
                return ap.rearrange("p (hp two) i -> p two hp i", two=2)[:, 0, :, :]

            def odd_heads(ap):
                return ap.rearrange("p (hp two) i -> p two hp i", two=2)[:, 1, :, :]


            nc.vector.memset(Vb[:, :, :, 64:65], 1.0)

            # ---- Phase 1: qkv^T for q,k; repack into Qpair/QT/KT ----------
            for ot in range(12):  # o-tiles: 0..5 -> q heads, 6..11 -> k heads
                ps = pspool.tile([128, 1024], F32, tag="ps")
                for ic in range(2):
                    for ct in range(6):
                        nc.tensor.matmul(
                            ps[:, ic * 512:(ic + 1) * 512],
                            wqkv[:, ct, ot * 128:(ot + 1) * 128],
                            xT[:, ct, ic * 512:(ic + 1) * 512],
                            start=(ct == 0), stop=(ct == 5),
                        )
                if ot < 6:
                    nc.any.tensor_scalar(
                        out=QT[0:64, 2 * ot, :], in0=ps[0:64, :],
                        scalar1=bqk[0:64, ot:ot + 1], scalar2=SCALE,
                        op0=OP.add, op1=OP.mult)
                    nc.any.tensor_scalar(
                        out=QT[64:128, 2 * ot + 1, :], in0=ps[64:128, :],
                        scalar1=bqk[64:128, ot:ot + 1], scalar2=SCALE,
                        op0=OP.add, op1=OP.mult)
                else:
                    h0 = (ot - 6) * 2
                    nc.any.tensor_scalar(
                        out=KT[0:64, h0, :], in0=ps[0:64, :],
                        scalar1=bqk[0:64, ot:ot + 1], scalar2=None, op0=OP.add)
                    nc.any.tensor_scalar(
                        out=KT[64:128, h0 + 1, :], in0=ps[64:128, :],
                        scalar1=bqk[64:128, ot:ot + 1], scalar2=None, op0=OP.add)

            # deferred const DMAs (not needed until phases 2+; keeps the
            # startup DMA queues dedicated to wqkv/xT)
            nc.sync.dma_start(out=rhT[0:64, :], in_=rhT_d[:])
            nc.sync.dma_start(out=rhT[64:128, :], in_=rhT_d[:])
            nc.sync.dma_start(out=rwT[0:64, :], in_=rwT_d[:])
            nc.sync.dma_start(out=rwT[64:128, :], in_=rwT_d[:])
            nc.sync.dma_start(out=even_heads(KT[64:96]), in_=eyeh_d[:])
            nc.sync.dma_start(out=odd_heads(KT[0:32]), in_=eyeh_d[:])
            nc.sync.dma_start(out=even_heads(KT[96:128]), in_=eyew_d[:])
            nc.sync.dma_start(out=odd_heads(KT[32:64]), in_=eyew_d[:])
            nc.sync.dma_start(out=wproj, in_=wproj_d[:])

            # ---- Phase 2: rel_h^T / rel_w^T, batched over heads per parity --
            # 4 ih per 2-bank psum tile (cols 0:192,192:384 | 512:704,704:896),
            # one merged strided copy per parity per group.
            def rel_src(pr, rows, bank, kind):
                # [128,1024] -> [32, 6, 2, 32]: (h, ihl, w|a) for one bank
                v = pr[:, bank * 512:bank * 512 + 384].rearrange(
                    "p (ihl hw) -> p ihl hw", ihl=2)
                if kind == "h":   # cols are (h, w)
                    v = v.rearrange("p ihl (h w) -> p h ihl w", w=32)
                else:             # cols are (h, a)
                    v = v.rearrange("p ihl (h a) -> p h ihl a", a=32)
                return v[rows[0]:rows[1]]

            for g in range(8):
                pr = pspool.tile([128, 1024], F32, tag="ps")
                for k in range(4):
                    ih = g * 4 + k
                    isl = slice(ih * 32, ih * 32 + 32)
                    col = (k // 2) * 512 + (k % 2) * 192
                    nc.tensor.matmul(pr[64:96, col:col + 192], rhT[0:64, isl],
                                     even_heads(QT[0:64])[:, :, isl],
                                     start=True, stop=True,
                                     tile_position=(0, 64))
                    nc.tensor.matmul(pr[0:32, col:col + 192], rhT[64:128, isl],
                                     odd_heads(QT[64:128])[:, :, isl],
                                     start=True, stop=True,
                                     tile_position=(64, 0))
                for bank in range(2):
                    csl = slice(g * 128 + bank * 64, g * 128 + bank * 64 + 64)
                    de = even_heads(QT[64:96])[:, :, csl].rearrange(
                        "p h (ihl w) -> p h ihl w", ihl=2)
                    do = odd_heads(QT[0:32])[:, :, csl].rearrange(
                        "p h (ihl w) -> p h ihl w", ihl=2)
                    nc.any.tensor_copy(out=de,
                                       in_=rel_src(pr, (64, 96), bank, "h"))
                    nc.any.tensor_copy(out=do,
                                       in_=rel_src(pr, (0, 32), bank, "h"))
            for g in range(8):
                pr = pspool.tile([128, 1024], F32, tag="ps")
                for k in range(4):
                    iw = g * 4 + k
                    wsl = slice(iw * 32, iw * 32 + 32)
                    col = (k // 2) * 512 + (k % 2) * 192
                    wc_e = even_heads(QT[0:64]).rearrange(
                        "p h (a b) -> p h a b", b=32)[:, :, :, iw]
                    wc_o = odd_heads(QT[64:128]).rearrange(
                        "p h (a b) -> p h a b", b=32)[:, :, :, iw]
                    nc.tensor.matmul(pr[96:128, col:col + 192], rwT[0:64, wsl],
                                     wc_e, start=True, stop=True,
                                     tile_position=(0, 96))
                    nc.tensor.matmul(pr[32:64, col:col + 192], rwT[64:128, wsl],
                                     wc_o, start=True, stop=True,
                                     tile_position=(64, 32))
                for bank in range(2):
                    iwsl = slice(g * 4 + bank * 2, g * 4 + bank * 2 + 2)
                    de = even_heads(QT[96:128]).rearrange(
                        "p h (a b) -> p h a b", b=32)[:, :, :, iwsl] \
                        .rearrange("p h a iwl -> p h iwl a")
                    do = odd_heads(QT[32:64]).rearrange(
                        "p h (a b) -> p h a b", b=32)[:, :, :, iwsl] \
                        .rearrange("p h a iwl -> p h iwl a")
                    nc.any.tensor_copy(out=de,
                                       in_=rel_src(pr, (96, 128), bank, "w"))
                    nc.any.tensor_copy(out=do,
                                       in_=rel_src(pr, (32, 64), bank, "w"))

            # ---- Phase 4 (interleaved with v): scores+exp / [v|1]@E ------
            def head_scores(h):
                E = epool.tile([128, 8, 1024], BF, tag="E")
                for jt in range(8):
                    ps = pspool.tile([128, 1024], F32, tag="ps")
                    for ic in range(2):
                        nc.tensor.matmul(ps[:, ic * 512:(ic + 1) * 512],
                                         KT[:, h, jt * 128:(jt + 1) * 128],
                                         QT[:, h, ic * 512:(ic + 1) * 512],
                                         start=True, stop=True)
                    nc.scalar.activation(out=E[:, jt, :], in_=ps, func=AF.Exp)
                return E

            def head_av(h, E, attn_outT):
                for ic in range(2):
                    pa = papool.tile([65, 512], F32, tag="pa")
                    for jt in range(8):
                        nc.tensor.matmul(pa[0:65, :], Vb[:, jt, h, 0:65],
                                         E[:, jt, ic * 512:(ic + 1) * 512],
                                         start=(jt == 0), stop=(jt == 7))
                    rec = divp.tile([65, 512], F32, tag="rec")
                    nc.vector.reciprocal(rec[64:65, :], pa[64:65, :])
                    # partition-broadcast via DRAM bounce (stride-0 partition
                    # APs are only legal on DRAM sources)
                    rd = drp.tile([1, 512], F32, tag="rd")
                    nc.sync.dma_start(out=rd, in_=rec[64:65, :])
                    bc = divp.tile([64, 512], F32, tag="bc")
                    nc.sync.dma_start(out=bc,
                                      in_=rd[0:1, :].to_broadcast([64, 512]))
                    nc.vector.tensor_mul(
                        attn_outT[:, h, ic * 512:(ic + 1) * 512],
                        pa[0:64, :], bc)

            Es = {0: head_scores(0), 1: head_scores(1)}
            # ---- Phase 3: v (non-transposed): v[n, (h,d)] -------------------
            for nt in range(8):
                for ovc in range(2):
                    pv = papool.tile([128, 384], F32, tag="pa")
                    for ct in range(6):
                        nc.tensor.matmul(
                            pv,
                            xT[:, ct, nt * 128:(nt + 1) * 128],
                            wqkv[:, ct, 1536 + ovc * 384:1536 + (ovc + 1) * 384],
                            start=(ct == 0), stop=(ct == 5),
                        )
                    src = pv.rearrange("p (h d) -> p h d", d=64)
                    nc.any.tensor_copy(
                        out=Vb[:, nt, ovc * 6:(ovc + 1) * 6, 0:64], in_=src)
            attn_outT = consts.tile([64, 12, 1024], BF, tag="big")
            for h in range(2, 12):
                Es[h] = head_scores(h)
                head_av(h - 2, Es.pop(h - 2), attn_outT)
            head_av(10, Es.pop(10), attn_outT)
            head_av(11, Es.pop(11), attn_outT)

            # ---- Phase 5: proj + b_eff --------------------------------------
            for cot in range(6):
                ps = pspool.tile([128, 1024], F32, tag="ps")
                for ic in range(2):
                    for h in range(12):
                        nc.tensor.matmul(ps[:, ic * 512:(ic + 1) * 512],
                                         wproj[:, h, cot * 128:(cot + 1) * 128],
                                         attn_outT[:, h, ic * 512:(ic + 1) * 512],
                                         start=(h == 0), stop=(h == 11))
                for ic in range(2):
                    osb = outp.tile([128, 512], F32, tag="osb")
                    nc.vector.tensor_scalar(
                        out=osb, in0=ps[:, ic * 512:(ic + 1) * 512],
                        scalar1=beff[:, cot:cot + 1], scalar2=None, op0=OP.add)
                    nc.sync.dma_start(
                        out=out_d[cot * 128:(cot + 1) * 128,
                                  ic * 512:(ic + 1) * 512],
                        in_=osb)

        for _rep in range(repeat):
            emit()

    nc.compile()
    return nc


def _get_nc():
    global _NC
    if _NC is None:
        _NC = _build()
    return _NC


def _prep_inputs(x, qkv_w, qkv_b, proj_w, proj_b, rel_pos_h, rel_pos_w):
    x = np.asarray(x, np.float32)
    qkv_w = np.asarray(qkv_w, np.float32)
    qkv_b = np.asarray(qkv_b, np.float32)
    proj_w = np.asarray(proj_w, np.float32)
    proj_b = np.asarray(proj_b, np.float32)
    rel_pos_h = np.asarray(rel_pos_h, np.float32)
    rel_pos_w = np.asarray(rel_pos_w, np.float32)

    wqkv = np.ascontiguousarray(
        qkv_w.T.reshape(6, 128, 3 * C).transpose(1, 0, 2)).astype(bf16)
    wproj = np.ascontiguousarray(
        proj_w.T.reshape(12, 64, C).transpose(1, 0, 2)).astype(bf16)
    bqk = np.ascontiguousarray(qkv_b[:2 * C].reshape(12, 128).T).astype(np.float32)
    beff = np.ascontiguousarray(
        (proj_w @ qkv_b[2 * C:] + proj_b).reshape(6, 128).T).astype(np.float32)

    coords = np.arange(32)[:, None] - np.arange(32)[None, :] + 31
    rhT = np.ascontiguousarray(
        rel_pos_h[coords].transpose(2, 0, 1).reshape(64, 1024)).astype(bf16)
    rwT = np.ascontiguousarray(
        rel_pos_w[coords].transpose(2, 0, 1).reshape(64, 1024)).astype(bf16)

    base_h = np.kron(np.eye(32, dtype=np.float32), np.ones((1, 32), np.float32))
    base_w = np.tile(np.eye(32, dtype=np.float32), (1, 32))
    eyeh = np.ascontiguousarray(
        np.broadcast_to(base_h[:, None, :], (32, 6, 1024))).astype(bf16)
    eyew = np.ascontiguousarray(
        np.broadcast_to(base_w[:, None, :], (32, 6, 1024))).astype(bf16)

    shared = dict(wqkv=wqkv, wproj=wproj, bqk=bqk, beff=beff,
                  rhT=rhT, rwT=rwT, eyeh=eyeh, eyew=eyew)
    in_maps = []
    for b in range(B):
        xT = np.ascontiguousarray(
            x[b].reshape(N, C).T.reshape(6, 128, N).transpose(1, 0, 2)
        ).astype(bf16)
        in_maps.append(dict(xT=xT, **shared))
    return in_maps


_last_results = None


def kernel(x, qkv_w, qkv_b, proj_w, proj_b, rel_pos_h, rel_pos_w):
    global _last_results
    from concourse.bass_utils import run_bass_kernel_spmd

    nc = _get_nc()
    in_maps = _prep_inputs(x, qkv_w, qkv_b, proj_w, proj_b,
                           rel_pos_h, rel_pos_w)
    res = run_bass_kernel_spmd(nc, in_maps, core_ids=list(range(8)))
    _last_results = res
    out = np.stack([
        np.asarray(res.results[b]["out"], np.float32).T.reshape(H, W, C)
        for b in range(B)
    ])
    return out


# revision 50
# speedup vs baseline: 1.0580x; 1.0580x over previous
"""ViTDet-style windowed attention w/ decomposed rel-pos, on 8 TRN2 NeuronCores.

Problem: x(8,32,32,768), 12 heads, hd=64, N=1024 tokens per image.
Sharding: pure data-parallel over B — core b handles image b; weights/tables
replicated; no collectives.

Per-core math (all matmuls bf16, fp32 PSUM accumulation):
  qkv^T[o, n]   = qkv_wT.T-chunks @ x^T          (o on partitions, n free)
  scores^T[j,i] = K'^T.T @ Q'^T   per head, where the 128-deep contraction is
                  [k(64) | onehot_jh(32) | onehot_jw(32)] x
                  [q_s(64) | rel_h^T(32) | rel_w^T(32)]
                  -> q.k + rel_h[i,jh] + rel_w[i,jw] in ONE matmul
  E = exp(scores^T)               (no max-subtraction: |scores| < 3)
  [out^T; rowsum] = [v | 1].T @ E (ones column gives softmax denominator free)
  attn_outT = out^T * (1/rowsum)  (broadcast via DRAM-bounce partition bcast)
  final^T = proj_wT.T @ attn_outT + b_eff,  b_eff = proj_w@b_v + proj_b (host)

Row-parity trick: qkv^T psum tiles hold head 2t on partitions 0:64 and head
2t+1 on 64:128; DVE cannot shift partitions, so even heads keep q/k on rows
0:64 (rel parts 64:128) while odd heads keep q/k on rows 64:128 (rel parts
0:64). The augmented inner product is row-permutation invariant per head.
Scores psum tiles are 2-bank [128,1024] so one exp covers two matmuls.
"""

import numpy as np
import ml_dtypes

bf16 = ml_dtypes.bfloat16

B, H, W, C = 8, 32, 32, 768
NH, HD = 12, 64
N = H * W  # 1024
SCALE = HD ** -0.5

_NC = None


def _build(repeat=1):
    import concourse.mybir as mybir
    import concourse.tile as tile
    from concourse import bacc

    BF = mybir.dt.bfloat16
    F32 = mybir.dt.float32
    AF = mybir.ActivationFunctionType
    OP = mybir.AluOpType

    nc = bacc.Bacc(None, target_bir_lowering=False)

    xT_d = nc.declare_dram_parameter("xT", [128, 6, 1024], BF, isOutput=False)
    wqkv_d = nc.declare_dram_parameter("wqkv", [128, 6, 2304], BF, isOutput=False)
    wproj_d = nc.declare_dram_parameter("wproj", [64, 12, 768], BF, isOutput=False)
    bqk_d = nc.declare_dram_parameter("bqk", [128, 12], F32, isOutput=False)
    beff_d = nc.declare_dram_parameter("beff", [128, 6], F32, isOutput=False)
    rhT_d = nc.declare_dram_parameter("rhT", [64, 1024], BF, isOutput=False)
    rwT_d = nc.declare_dram_parameter("rwT", [64, 1024], BF, isOutput=False)
    eyeh_d = nc.declare_dram_parameter("eyeh", [32, 6, 1024], BF, isOutput=False)
    eyew_d = nc.declare_dram_parameter("eyew", [32, 6, 1024], BF, isOutput=False)
    out_d = nc.declare_dram_parameter("out", [768, 1024], F32, isOutput=True)

    with tile.TileContext(nc) as tc, \
            tc.tile_pool(name="consts", bufs=1) as consts, \
            tc.tile_pool(name="epool", bufs=3) as epool, \
            tc.tile_pool(name="ps", bufs=2, space="PSUM") as pspool, \
            tc.tile_pool(name="pa", bufs=4, space="PSUM") as papool, \
            tc.tile_pool(name="divp", bufs=3) as divp, \
            tc.tile_pool(name="drp", bufs=4, space="DRAM") as drp, \
            tc.tile_pool(name="outp", bufs=2) as outp:

        def emit():
            # xT shares its SBUF slot with attn_outT (tag "big"): xT dies
            # after phase 3, attn_outT is born in phase 4.
            xT = consts.tile([128, 6, 1024], BF, tag="big")
            wqkv = consts.tile([128, 6, 2304], BF)
            for ct in range(6):
                nc.sync.dma_start(out=xT[:, ct, :], in_=xT_d[:, ct, :])
                for third in range(3):
                    osl = slice(third * 768, (third + 1) * 768)
                    nc.sync.dma_start(out=wqkv[:, ct, osl],
                                      in_=wqkv_d[:, ct, osl])
            wproj = consts.tile([64, 12, 768], BF)
            bqk = consts.tile([128, 12], F32)
            nc.sync.dma_start(out=bqk, in_=bqk_d[:])
            beff = consts.tile([128, 6], F32)
            nc.sync.dma_start(out=beff, in_=beff_d[:])
            # DVE "touch" of DMA-loaded scalars: absorbs the DMA-lane
            # semaphore waits so downstream tensor_scalar ops need only the
            # PE wait (fewer event-semaphore splits).
            tch = consts.tile([1, 18], F32)
            nc.vector.tensor_copy(out=tch[:, 0:12], in_=bqk[0:1, :])
            nc.vector.tensor_copy(out=tch[:, 12:18], in_=beff[0:1, :])
            # rel-pos tables duplicated on both partition halves so odd-parity
            # matmuls (operands on partitions 64:128) have an aligned lhsT.
            rhT = consts.tile([128, 1024], BF)
            rwT = consts.tile([128, 1024], BF)

            KT = consts.tile([128, 12, 1024], BF)   # augmented K'^T per head
            QT = consts.tile([128, 12, 1024], BF)   # augmented Q'^T per head
            Vb = consts.tile([128, 8, 12, 65], BF)  # [n-part, jt, head, v|1]

            def even_heads(ap):  # [p, 12, i] -> [p, 6, i] (heads 0,2,..)
                return ap.rearrange("p (hp two) i -> p two hp i", two=2)[:, 0, :, :]

            def odd_heads(ap):
                return ap.rearrange("p (hp two) i -> p two hp i", two=2)[:, 1, :, :]


            nc.vector.memset(Vb[:, :, :, 64:65], 1.0)
            ones_r = consts.tile([65, 64], mybir.dt.float32r)
            ones_f = consts.tile([65, 64], F32)
            nc.vector.memset(ones_f[64:65, :], 1.0)
            nc.vector.tensor_copy(out=ones_r[64:65, :], in_=ones_f[64:65, :])

            # ---- Phase 1: qkv^T for q,k; repack into Qpair/QT/KT ----------
            for ot in range(12):  # o-tiles: 0..5 -> q heads, 6..11 -> k heads
                ps = pspool.tile([128, 1024], F32, tag="ps")
                for ic in range(2):
                    for ct in range(6):
                        nc.tensor.matmul(
                            ps[:, ic * 512:(ic + 1) * 512],
                            wqkv[:, ct, ot * 128:(ot + 1) * 128],
                            xT[:, ct, ic * 512:(ic + 1) * 512],
                            start=(ct == 0), stop=(ct == 5),
                        )
                if ot < 6:
                    nc.any.tensor_scalar(
                        out=QT[0:64, 2 * ot, :], in0=ps[0:64, :],
                        scalar1=bqk[0:64, ot:ot + 1], scalar2=SCALE,
                        op0=OP.add, op1=OP.mult)
                    nc.any.tensor_scalar(
                        out=QT[64:128, 2 * ot + 1, :], in0=ps[64:128, :],
                        scalar1=bqk[64:128, ot:ot + 1], scalar2=SCALE,
                        op0=OP.add, op1=OP.mult)
                else:
                    h0 = (ot - 6) * 2
                    nc.any.tensor_scalar(
                        out=KT[0:64, h0, :], in0=ps[0:64, :],
                        scalar1=bqk[0:64, ot:ot + 1], scalar2=None, op0=OP.add)
                    nc.any.tensor_scalar(
                        out=KT[64:128, h0 + 1, :], in0=ps[64:128, :],
                        scalar1=bqk[64:128, ot:ot + 1], scalar2=None, op0=OP.add)

            # deferred const DMAs (not needed until phases 2+; keeps the
            # startup DMA queues dedicated to wqkv/xT)
            nc.sync.dma_start(out=rhT[0:64, :], in_=rhT_d[:])
            nc.sync.dma_start(out=rhT[64:128, :], in_=rhT_d[:])
            nc.sync.dma_start(out=rwT[0:64, :], in_=rwT_d[:])
            nc.sync.dma_start(out=rwT[64:128, :], in_=rwT_d[:])
            nc.sync.dma_start(out=even_heads(KT[64:96]), in_=eyeh_d[:])
            nc.sync.dma_start(out=odd_heads(KT[0:32]), in_=eyeh_d[:])
            nc.sync.dma_start(out=even_heads(KT[96:128]), in_=eyew_d[:])
            nc.sync.dma_start(out=odd_heads(KT[32:64]), in_=eyew_d[:])
            nc.sync.dma_start(out=wproj, in_=wproj_d[:])

            # ---- Phase 2: rel_h^T / rel_w^T, batched over heads per parity --
            # 4 ih per 2-bank psum tile (cols 0:192,192:384 | 512:704,704:896),
            # one merged strided copy per parity per group.
            def rel_src(pr, rows, bank, kind):
                # [128,1024] -> [32, 6, 2, 32]: (h, ihl, w|a) for one bank
                v = pr[:, bank * 512:bank * 512 + 384].rearrange(
                    "p (ihl hw) -> p ihl hw", ihl=2)
                if kind == "h":   # cols are (h, w)
                    v = v.rearrange("p ihl (h w) -> p h ihl w", w=32)
                else:             # cols are (h, a)
                    v = v.rearrange("p ihl (h a) -> p h ihl a", a=32)
                return v[rows[0]:rows[1]]

            for g in range(8):
                pr = pspool.tile([128, 1024], F32, tag="ps")
                for k in range(4):
                    ih = g * 4 + k
                    isl = slice(ih * 32, ih * 32 + 32)
                    col = (k // 2) * 512 + (k % 2) * 192
                    nc.tensor.matmul(pr[64:96, col:col + 192], rhT[0:64, isl],
                                     even_heads(QT[0:64])[:, :, isl],
                                     start=True, stop=True,
                                     tile_position=(0, 64))
                    nc.tensor.matmul(pr[0:32, col:col + 192], rhT[64:128, isl],
                                     odd_heads(QT[64:128])[:, :, isl],
                                     start=True, stop=True,
                                     tile_position=(64, 0))
                for bank in range(2):
                    csl = slice(g * 128 + bank * 64, g * 128 + bank * 64 + 64)
                    de = even_heads(QT[64:96])[:, :, csl].rearrange(
                        "p h (ihl w) -> p h ihl w", ihl=2)
                    do = odd_heads(QT[0:32])[:, :, csl].rearrange(
                        "p h (ihl w) -> p h ihl w", ihl=2)
                    nc.any.tensor_copy(out=de,
                                       in_=rel_src(pr, (64, 96), bank, "h"))
                    nc.any.tensor_copy(out=do,
                                       in_=rel_src(pr, (0, 32), bank, "h"))
            for g in range(8):
                pr = pspool.tile([128, 1024], F32, tag="ps")
                for k in range(4):
                    iw = g * 4 + k
                    wsl = slice(iw * 32, iw * 32 + 32)
                    col = (k // 2) * 512 + (k % 2) * 192
                    wc_e = even_heads(QT[0:64]).rearrange(
                        "p h (a b) -> p h a b", b=32)[:, :, :, iw]
                    wc_o = odd_heads(QT[64:128]).rearrange(
                        "p h (a b) -> p h a b", b=32)[:, :, :, iw]
                    nc.tensor.matmul(pr[96:128, col:col + 192], rwT[0:64, wsl],
                                     wc_e, start=True, stop=True,
                                     tile_position=(0, 96))
                    nc.tensor.matmul(pr[32:64, col:col + 192], rwT[64:128, wsl],
                                     wc_o, start=True, stop=True,
                                     tile_position=(64, 32))
                for bank in range(2):
                    iwsl = slice(g * 4 + bank * 2, g * 4 + bank * 2 + 2)
                    de = even_heads(QT[96:128]).rearrange(
                        "p h (a b) -> p h a b", b=32)[:, :, :, iwsl] \
                        .rearrange("p h a iwl -> p h iwl a")
                    do = odd_heads(QT[32:64]).rearrange(
                        "p h (a b) -> p h a b", b=32)[:, :, :, iwsl] \
                        .rearrange("p h a iwl -> p h iwl a")
                    nc.any.tensor_copy(out=de,
                                       in_=rel_src(pr, (96, 128), bank, "w"))
                    nc.any.tensor_copy(out=do,
                                       in_=rel_src(pr, (32, 64), bank, "w"))

            # ---- Phase 4 (interleaved with v): scores+exp / [v|1]@E ------
            def head_scores(h):
                E = epool.tile([128, 8, 1024], BF, tag="E")
                for jt in range(8):
                    ps = pspool.tile([128, 1024], F32, tag="ps")
                    for ic in range(2):
                        nc.tensor.matmul(ps[:, ic * 512:(ic + 1) * 512],
                                         KT[:, h, jt * 128:(jt + 1) * 128],
                                         QT[:, h, ic * 512:(ic + 1) * 512],
                                         start=True, stop=True)
                    nc.scalar.activation(out=E[:, jt, :], in_=ps, func=AF.Exp)
                return E

            def head_av(h, E, attn_outT):
                for ic in range(2):
                    pa = papool.tile([65, 512], F32, tag="pa")
                    for jt in range(8):
                        nc.tensor.matmul(pa[0:65, :], Vb[:, jt, h, 0:65],
                                         E[:, jt, ic * 512:(ic + 1) * 512],
                                         start=(jt == 0), stop=(jt == 7))
                    rec = divp.tile([65, 512], mybir.dt.float32r, tag="rec")
                    with nc.allow_low_precision(reason="f32r is 19-bit; fine"):
                        nc.vector.reciprocal(rec[64:65, :], pa[64:65, :])
                    # partition-broadcast via a K=1 f32r matmul (full rate at
                    # N=512); avoids the DRAM-bounce DMA round trip.
                    bcp = papool.tile([64, 512], F32, tag="pa")
                    nc.tensor.matmul(bcp, ones_r[64:65, :], rec[64:65, :],
                                     start=True, stop=True)
                    bc = divp.tile([64, 512], F32, tag="bc")
                    nc.vector.tensor_copy(out=bc, in_=bcp)
                    nc.vector.tensor_mul(
                        attn_outT[:, h, ic * 512:(ic + 1) * 512],
                        pa[0:64, :], bc)

            Es = {0: head_scores(0), 1: head_scores(1)}
            # ---- Phase 3: v (non-transposed): v[n, (h,d)] -------------------
            for nt in range(8):
                for ovc in range(2):
                    pv = papool.tile([128, 384], F32, tag="pa")
                    for ct in range(6):
                        nc.tensor.matmul(
                            pv,
                            xT[:, ct, nt * 128:(nt + 1) * 128],
                            wqkv[:, ct, 1536 + ovc * 384:1536 + (ovc + 1) * 384],
                            start=(ct == 0), stop=(ct == 5),
                        )
                    src = pv.rearrange("p (h d) -> p h d", d=64)
                    nc.any.tensor_copy(
                        out=Vb[:, nt, ovc * 6:(ovc + 1) * 6, 0:64], in_=src)
            attn_outT = consts.tile([64, 12, 1024], BF, tag="big")
            for h in range(2, 12):
                Es[h] = head_scores(h)
                head_av(h - 2, Es.pop(h - 2), attn_outT)
            head_av(10, Es.pop(10), attn_outT)
            head_av(11, Es.pop(11), attn_outT)

            # ---- Phase 5: proj + b_eff --------------------------------------
            for cot in range(6):
                ps = pspool.tile([128, 1024], F32, tag="ps")
                for ic in range(2):
                    for h in range(12):
                        nc.tensor.matmul(ps[:, ic * 512:(ic + 1) * 512],
                                         wproj[:, h, cot * 128:(cot + 1) * 128],
                                         attn_outT[:, h, ic * 512:(ic + 1) * 512],
                                         start=(h == 0), stop=(h == 11))
                for ic in range(2):
                    osb = outp.tile([128, 512], F32, tag="osb")
                    nc.vector.tensor_scalar(
                        out=osb, in0=ps[:, ic * 512:(ic + 1) * 512],
                        scalar1=beff[:, cot:cot + 1], scalar2=None, op0=OP.add)
                    nc.sync.dma_start(
                        out=out_d[cot * 128:(cot + 1) * 128,
                                  ic * 512:(ic + 1) * 512],
                        in_=osb)

        for _rep in range(repeat):
            emit()

    nc.compile()
    return nc


def _get_nc():
    global _NC
    if _NC is None:
        _NC = _build()
    return _NC


def _prep_inputs(x, qkv_w, qkv_b, proj_w, proj_b, rel_pos_h, rel_pos_w):
    x = np.asarray(x, np.float32)
    qkv_w = np.asarray(qkv_w, np.float32)
    qkv_b = np.asarray(qkv_b, np.float32)
    proj_w = np.asarray(proj_w, np.float32)
    proj_b = np.asarray(proj_b, np.float32)
    rel_pos_h = np.asarray(rel_pos_h, np.float32)
    rel_pos_w = np.asarray(rel_pos_w, np.float32)

    wqkv = np.ascontiguousarray(
        qkv_w.T.reshape(6, 128, 3 * C).transpose(1, 0, 2)).astype(bf16)
    wproj = np.ascontiguousarray(
        proj_w.T.reshape(12, 64, C).transpose(1, 0, 2)).astype(bf16)
    bqk = np.ascontiguousarray(qkv_b[:2 * C].reshape(12, 128).T).astype(np.float32)
    beff = np.ascontiguousarray(
        (proj_w @ qkv_b[2 * C:] + proj_b).reshape(6, 128).T).astype(np.float32)

    coords = np.arange(32)[:, None] - np.arange(32)[None, :] + 31
    rhT = np.ascontiguousarray(
        rel_pos_h[coords].transpose(2, 0, 1).reshape(64, 1024)).astype(bf16)
    rwT = np.ascontiguousarray(
        rel_pos_w[coords].transpose(2, 0, 1).reshape(64, 1024)).astype(bf16)

    base_h = np.kron(np.eye(32, dtype=np.float32), np.ones((1, 32), np.float32))
    base_w = np.tile(np.eye(32, dtype=np.float32), (1, 32))
    eyeh = np.ascontiguousarray(
        np.broadcast_to(base_h[:, None, :], (32, 6, 1024))).astype(bf16)
    eyew = np.ascontiguousarray(
        np.broadcast_to(base_w[:, None, :], (32, 6, 1024))).astype(bf16)

    shared = dict(wqkv=wqkv, wproj=wproj, bqk=bqk, beff=beff,
                  rhT=rhT, rwT=rwT, eyeh=eyeh, eyew=eyew)
    in_maps = []
    for b in range(B):
        xT = np.ascontiguousarray(
            x[b].reshape(N, C).T.reshape(6, 128, N).transpose(1, 0, 2)
        ).astype(bf16)
        in_maps.append(dict(xT=xT, **shared))
    return in_maps


_last_results = None


def kernel(x, qkv_w, qkv_b, proj_w, proj_b, rel_pos_h, rel_pos_w):
    global _last_results
    from concourse.bass_utils import run_bass_kernel_spmd

    nc = _get_nc()
    in_maps = _prep_inputs(x, qkv_w, qkv_b, proj_w, proj_b,
                           rel_pos_h, rel_pos_w)
    res = run_bass_kernel_spmd(nc, in_maps, core_ids=list(range(8)))
    _last_results = res
    out = np.stack([
        np.asarray(res.results[b]["out"], np.float32).T.reshape(H, W, C)
        for b in range(B)
    ])
    return out
